# revision 9
# baseline (speedup 1.0000x reference)
"""Adaptive-threshold spike encoding on 8 TRN2 NeuronCores.

Math: the reference iterates, per element with input value x > 0:
    acc += x; spike = acc >= thr; acc = 0 where spike; thr = 0.9*thr + 0.1*|x|
Since thr's recurrence is independent of the spikes, thr_t = A_t + B_t*x with
A_t = 0.5*0.9^t, B_t = 1-0.9^t.  Between resets acc = k*x (k = steps since
last fire), so the fire test  k*x >= A_t + B_t*x  <=>  k >= A_t*(1/x) + B_t.
With z = 1/x (computed host-side, exact fp32) and the 0-based counter
k* = k-1, each timestep needs only:
    ck = z*A_t + (B_t - 1)        (tensor_scalar, VectorE 2x fp32)
    sn = (k* < ck)                (tensor_tensor is_lt -> 1.0/0.0)
    spike = 1 - sn                (ScalarE activation Copy, scale=-1 bias=1)
    k* = (k* + 1) * sn            (scalar_tensor_tensor)
Embarrassingly data-parallel: batch 32 -> 4 per core across 8 cores.
"""

import numpy as np

import concourse.bacc as bacc
import concourse.bass as bass
import concourse.mybir as mybir
from concourse.tile import TileContext
from concourse.bass_utils import run_bass_kernel_spmd

TIMESTEPS = 32
N_CORES = 8
B_FULL = 32
B_CORE = B_FULL // N_CORES  # 4 batches per core
P = 128
FD = (256 * 1024) // P  # 2048: one [128, 2048] tile = one batch's features

FP32 = mybir.dt.float32
BF16 = mybir.dt.bfloat16
Alu = mybir.AluOpType


def _betas():
    betas = []
    b = 0.0
    for _ in range(TIMESTEPS):
        betas.append(float(b - 1.0))
        b = 0.9 * b + 0.1
    return betas


def _build_nc(k_engine=None, zt_engine=None) -> bass.Bass:
    """Per (t, b) step-chunk the ops are:
        sn = (k - beta_t) < zt     scalar_tensor_tensor   [VectorE]
        spike = 1 - sn             activation Copy        [ScalarE]
        k = (k + 1) * sn           scalar_tensor_tensor   [k_engine(t,b)]
        zt = zt * 0.9              tensor_scalar_mul      [zt_engine(t,b)]
    k_engine/zt_engine: callables (t, b) -> engine name for load balancing.
    """
    nc = bacc.Bacc()
    z_ext = nc.declare_dram_parameter("z", [B_CORE, P, FD], FP32, isOutput=False)
    out_ext = nc.declare_dram_parameter(
        "out", [B_CORE, TIMESTEPS, P, FD], FP32, isOutput=True
    )
    betas = _betas()
    k_engine = k_engine or (lambda t, b: "vector")
    zt_engine = zt_engine or (lambda t, b: "gpsimd")

    def eng(name):
        return getattr(nc, name)

    with TileContext(nc) as tc:
        with (
            tc.tile_pool(name="state", bufs=1) as state_pool,
            tc.tile_pool(name="work", bufs=4) as work_pool,
            tc.tile_pool(name="outp", bufs=6) as out_pool,
        ):
            zt_tiles, k_tiles = [], []
            for b in range(B_CORE):
                zt = state_pool.tile([P, FD], FP32, tag=f"z{b}")
                nc.sync.dma_start(out=zt[:], in_=z_ext[b])
                k_t = state_pool.tile([P, FD], BF16, tag=f"k{b}")
                nc.vector.memset(k_t[:], 0.0)
                zt_tiles.append(zt)
                k_tiles.append(k_t)

            for t in range(TIMESTEPS):
                bm_t = betas[t]
                last = t == TIMESTEPS - 1
                for b in range(B_CORE):
                    sn = work_pool.tile([P, FD], BF16, tag="sn")
                    nc.vector.scalar_tensor_tensor(
                        sn[:], k_tiles[b][:], bm_t, zt_tiles[b][:],
                        Alu.subtract, Alu.is_lt,
                    )
                    spike = out_pool.tile([P, FD], FP32, tag="spk")
                    nc.scalar.activation(
                        spike[:],
                        sn[:],
                        mybir.ActivationFunctionType.Copy,
                        bias=1.0,
                        scale=-1.0,
                    )
                    if not last:
                        eng(k_engine(t, b)).scalar_tensor_tensor(
                            k_tiles[b][:], k_tiles[b][:], 1.0, sn[:],
                            Alu.add, Alu.mult,
                        )
                        ze = zt_engine(t, b)
                        if ze == "scalar":
                            nc.scalar.activation(
                                zt_tiles[b][:], zt_tiles[b][:],
                                mybir.ActivationFunctionType.Copy,
                                bias=0.0, scale=0.9,
                            )
                        else:
                            eng(ze).tensor_scalar_mul(
                                zt_tiles[b][:], zt_tiles[b][:], 0.9
                            )
                    nc.sync.dma_start(out=out_ext[b, t], in_=spike[:])
    nc.finalize()
    return nc


def kernel(x: np.ndarray, _profile: list | None = None) -> np.ndarray:
    assert x.shape == (B_FULL, 256, 1024), x.shape
    x = np.ascontiguousarray(x, dtype=np.float32)
    with np.errstate(divide="ignore"):
        z = (np.float32(0.5) / x).astype(np.float32)
    z = z.reshape(N_CORES, B_CORE, P, FD)

    nc = _build_nc()
    in_maps = [{"z": np.ascontiguousarray(z[i])} for i in range(N_CORES)]
    res = run_bass_kernel_spmd(nc, in_maps, core_ids=list(range(N_CORES)))
    if _profile is not None:
        _profile.append(res)

    out = np.empty((B_FULL, TIMESTEPS, 256, 1024), dtype=np.float32)
    for i in range(N_CORES):
        out[i * B_CORE : (i + 1) * B_CORE] = res.results[i]["out"].reshape(
            B_CORE, TIMESTEPS, 256, 1024
        )
    return out


# revision 11
# speedup vs baseline: 6.9622x; 6.9622x over previous
"""Adaptive-threshold spike encoding on 8 TRN2 NeuronCores.

Math: the reference iterates, per element with input value x > 0:
    acc += x; spike = acc >= thr; acc = 0 where spike; thr = 0.9*thr + 0.1*|x|
Since thr's recurrence is independent of the spikes, thr_t = A_t + B_t*x with
A_t = 0.5*0.9^t, B_t = 1-0.9^t.  Between resets acc = k*x (k = steps since
last fire), so the fire test  k*x >= A_t + B_t*x  <=>  k >= A_t*(1/x) + B_t.
With z = 1/x (computed host-side, exact fp32) and the 0-based counter
k* = k-1, each timestep needs only:
    ck = z*A_t + (B_t - 1)        (tensor_scalar, VectorE 2x fp32)
    sn = (k* < ck)                (tensor_tensor is_lt -> 1.0/0.0)
    spike = 1 - sn                (ScalarE activation Copy, scale=-1 bias=1)
    k* = (k* + 1) * sn            (scalar_tensor_tensor)
Embarrassingly data-parallel: batch 32 -> 4 per core across 8 cores.
"""

import numpy as np

import concourse.bacc as bacc
import concourse.bass as bass
import concourse.mybir as mybir
from concourse.tile import TileContext
from concourse.bass_utils import run_bass_kernel_spmd

TIMESTEPS = 32
N_CORES = 8
B_FULL = 32
B_CORE = B_FULL // N_CORES  # 4 batches per core
P = 128
FD = (256 * 1024) // P  # 2048: one [128, 2048] tile = one batch's features

FP32 = mybir.dt.float32
BF16 = mybir.dt.bfloat16
Alu = mybir.AluOpType


def _betas():
    betas = []
    b = 0.0
    for _ in range(TIMESTEPS):
        betas.append(float(b - 1.0))
        b = 0.9 * b + 0.1
    return betas


def _build_nc(k_engine=None, zt_engine=None) -> bass.Bass:
    """Per (t, b) step-chunk the ops are:
        sn = (k - beta_t) < zt     scalar_tensor_tensor   [VectorE]
        spike = 1 - sn             activation Copy        [ScalarE]
        k = (k + 1) * sn           scalar_tensor_tensor   [k_engine(t,b)]
        zt = zt * 0.9              tensor_scalar_mul      [zt_engine(t,b)]
    k_engine/zt_engine: callables (t, b) -> engine name for load balancing.
    """
    nc = bacc.Bacc()
    z_ext = nc.declare_dram_parameter("z", [B_CORE, P, FD], FP32, isOutput=False)
    out_ext = nc.declare_dram_parameter(
        "out", [B_CORE, TIMESTEPS, P, FD], FP32, isOutput=True
    )
    betas = _betas()
    k_engine = k_engine or (lambda t, b: "vector")
    zt_engine = zt_engine or (lambda t, b: "scalar")

    def eng(name):
        return getattr(nc, name)

    with TileContext(nc) as tc:
        with (
            tc.tile_pool(name="state", bufs=1) as state_pool,
            tc.tile_pool(name="work", bufs=4) as work_pool,
            tc.tile_pool(name="outp", bufs=6) as out_pool,
        ):
            zt_tiles, k_tiles = [], []
            for b in range(B_CORE):
                zt = state_pool.tile([P, FD], FP32, tag=f"z{b}")
                nc.sync.dma_start(out=zt[:], in_=z_ext[b])
                k_t = state_pool.tile([P, FD], FP32, tag=f"k{b}")
                nc.vector.memset(k_t[:], 0.0)
                zt_tiles.append(zt)
                k_tiles.append(k_t)

            for t in range(TIMESTEPS):
                bm_t = betas[t]
                last = t == TIMESTEPS - 1
                for b in range(B_CORE):
                    sn = work_pool.tile([P, FD], FP32, tag="sn")
                    nc.vector.scalar_tensor_tensor(
                        sn[:], k_tiles[b][:], bm_t, zt_tiles[b][:],
                        Alu.subtract, Alu.is_lt,
                    )
                    spike = out_pool.tile([P, FD], FP32, tag="spk")
                    nc.scalar.activation(
                        spike[:],
                        sn[:],
                        mybir.ActivationFunctionType.Copy,
                        bias=1.0,
                        scale=-1.0,
                    )
                    if not last:
                        eng(k_engine(t, b)).scalar_tensor_tensor(
                            k_tiles[b][:], k_tiles[b][:], 1.0, sn[:],
                            Alu.add, Alu.mult,
                        )
                        ze = zt_engine(t, b)
                        if ze == "scalar":
                            nc.scalar.activation(
                                zt_tiles[b][:], zt_tiles[b][:],
                                mybir.ActivationFunctionType.Copy,
                                bias=0.0, scale=0.9,
                            )
                        else:
                            eng(ze).tensor_scalar_mul(
                                zt_tiles[b][:], zt_tiles[b][:], 0.9
                            )
                    nc.sync.dma_start(out=out_ext[b, t], in_=spike[:])
    nc.finalize()
    return nc


def kernel(x: np.ndarray, _profile: list | None = None) -> np.ndarray:
    assert x.shape == (B_FULL, 256, 1024), x.shape
    x = np.ascontiguousarray(x, dtype=np.float32)
    with np.errstate(divide="ignore"):
        z = (np.float32(0.5) / x).astype(np.float32)
    z = z.reshape(N_CORES, B_CORE, P, FD)

    nc = _build_nc()
    in_maps = [{"z": np.ascontiguousarray(z[i])} for i in range(N_CORES)]
    res = run_bass_kernel_spmd(nc, in_maps, core_ids=list(range(N_CORES)))
    if _profile is not None:
        _profile.append(res)

    out = np.empty((B_FULL, TIMESTEPS, 256, 1024), dtype=np.float32)
    for i in range(N_CORES):
        out[i * B_CORE : (i + 1) * B_CORE] = res.results[i]["out"].reshape(
            B_CORE, TIMESTEPS, 256, 1024
        )
    return out
